# revision 30
# baseline (speedup 1.0000x reference)
"""Trainium2 Bass kernel for nn_AttentionModule (dense_transformer).

Reference computation (per batch sample b):
    theta = sigmoid(x @ Wt + bt)            # [N, F]
    phi   = x @ Wp + bp                     # [N, F]
    att   = theta @ phi.T                   # [N(n), N(m)]
    att   = softmax(att, axis over n)       # softmax over QUERY axis
    out   = att(n,m) @ x(m,d) + x           # [N, D]
  (the g = tanh(x@Wg+bg) branch is dead — never used in the output)

Strategy: pure data parallelism — B=8 samples, one per NeuronCore. No
collectives. Per core, everything runs in transposed score layout
ST[m, n] = phi[m]·theta[n] so the softmax axis (n) is the free axis;
softmax runs without max-subtraction (logits < ~60, exp(ST-20) is
fp32-safe and the shift cancels in the normalization).

ALL matmuls use fp8e4 DoubleRow (0.5 PE cycles/output column — 4x the
bf16 rate; each instruction contracts a pair of 128-deep k-tiles).
Accuracy comes from hi+lo operand splitting (x_lo = x - fp8(x)):
 - projections: 3 terms (Wh·xh + Wh·xl + Wl·xh), W pre-scaled by 32 on
   the host so W*32 ~ N(0,1) avoids the fp8 subnormal floor; the /32
   rides the activation `scale` input. Splits are free (host-side).
 - scores: 3 terms with theta/phi split on-device (Act/Pool copy for
   hi, DVE scalar_tensor_tensor for lo), tiled per (fc, ns) so the
   splits pipeline behind the projections.
 - phase 3: A = E/s quantized to single fp8 (~1.3e-2, the dominant
   error term), x split hi+lo on the host (2 terms).
 - residual in bf16, output stored bf16 (harness converts to f32).
Measured end-to-end rel err ~1.4e-2 (gate: 2e-2), on hw and CoreSim.

Phase structure per core:
 P1: thetaT/phiT [f, n]: per (ns, fc): 12 DoubleRow matmuls into one
     bank of a 4-bank PSUM tile; sigmoid (scale=1/32, bias bt) / DVE
     tensor_scalar (mult 1/32, add bp); then per (fc, ns) hi/lo split
     ops writing fp8 pair tiles.
 P2: per m-chunk: 24 DoubleRow matmuls -> ST [128, 2048] in 4 PSUM
     banks; ONE exp activation (bias -20) -> E bf16 rotating tile,
     accum_out gives the row-sum free; reciprocal; tensor_scalar_mul
     E*(1/s) -> A fp8 pair tiles (alternating DVE/Pool).
 P3: per n-chunk-pair: 4 accumulation groups in one 4-bank PSUM tile,
     16 DoubleRow matmuls each (8 m-pairs x {xn_hi, xn_lo}); DVE adds
     the bf16 residual; DMA out bf16.

Scheduling notes (walrus sync-wait limits + Tile dep granularity):
 - built as bacc.Bacc: finalize() legalizes multi-sem waits;
 - every SBUF tile is written by exactly ONE dma_start, and tiles are
   split to match consumer granularity (deps are tile-granular);
 - SBUF pools never overlap/reuse address space;
 - PE warm-up: dummy matmuls burn the initial DMA-wait so the real
   stream starts at 2.4GHz.
"""

import numpy as np
import ml_dtypes

import concourse.bass as bass
import concourse.bacc as bacc
import concourse.mybir as mybir
from concourse.tile import TileContext
from concourse.bass_utils import run_bass_kernel_spmd

P = 128
B, N, D, F = 8, 2048, 1024, 512
NCH = N // P    # 16 chunks of the token dim
NPR = NCH // 2  # 8 m-chunk pairs (DoubleRow granularity)
DCH = D // P    # 8 chunks of the model dim
DPR = DCH // 2  # 4 d-chunk pairs
FCH = F // P    # 4 chunks of the filter dim
FPR = FCH // 2  # 2 f-chunk pairs
NF = 512        # matmul moving free dim (one fp32 PSUM bank)
NSL = N // NF   # 4 score column slices
DSL = D // NF   # 2 output d slices
WSC = 32.0      # host pre-scale of W (keeps W*32 out of fp8 subnormals)

BF16 = mybir.dt.bfloat16
FP8 = mybir.dt.float8e4
F32 = mybir.dt.float32
AF = mybir.ActivationFunctionType
OP = mybir.AluOpType
DR = mybir.MatmulPerfMode.DoubleRow


def build_bass():
    nc = bacc.Bacc()

    # x.T in DoubleRow pair layout, hi/lo fp8 streams, ns-major for
    # contiguous per-ns DMAs: [p, ns, dp, i, no] = x[ns*512+no, (2dp+i)*128+p]
    xth_d = nc.declare_dram_parameter("xth", [P, NSL, DPR, 2, NF], FP8, isOutput=False)
    xtl_d = nc.declare_dram_parameter("xtl", [P, NSL, DPR, 2, NF], FP8, isOutput=False)
    # phase-3 moving streams: [p, j, i, d] = x[(2j+i)*128+p, d]
    xnh_d = nc.declare_dram_parameter("xnh", [P, NPR, 2, D], FP8, isOutput=False)
    xnl_d = nc.declare_dram_parameter("xnl", [P, NPR, 2, D], FP8, isOutput=False)
    xr_d = nc.declare_dram_parameter("xr", [N, D], BF16, isOutput=False)
    # weights (pre-scaled by 32) in pair layout:
    # [p, fc, dp, i, fo] = 32*W[(2dp+i)*128+p, fc*128+fo]
    wth_d = nc.declare_dram_parameter("wth", [P, FCH, DPR, 2, P], FP8, isOutput=False)
    wtl_d = nc.declare_dram_parameter("wtl", [P, FCH, DPR, 2, P], FP8, isOutput=False)
    wph_d = nc.declare_dram_parameter("wph", [P, FCH, DPR, 2, P], FP8, isOutput=False)
    wpl_d = nc.declare_dram_parameter("wpl", [P, FCH, DPR, 2, P], FP8, isOutput=False)
    bt_d = nc.declare_dram_parameter("bt", [P, FCH], F32, isOutput=False)
    bp_d = nc.declare_dram_parameter("bp", [P, FCH], F32, isOutput=False)
    out_d = nc.declare_dram_parameter("out", [N, D], BF16, isOutput=True)

    with TileContext(nc) as tc:
        with (
            tc.tile_pool(name="const", bufs=1) as cpool,
            tc.tile_pool(name="mid", bufs=1) as mid,
            tc.tile_pool(name="apool", bufs=1) as apool,
            tc.tile_pool(name="erot", bufs=2) as erot,
            tc.tile_pool(name="stats", bufs=16) as stats,
            tc.tile_pool(name="xst", bufs=2) as xstp,
            tc.tile_pool(name="ost", bufs=2) as ostp,
            tc.tile_pool(name="ozt", bufs=3) as oztp,
            tc.tile_pool(name="psum4", bufs=2, space="PSUM") as psum4,
        ):
            # --- constant/streamed input tiles (one DMA each) ---
            wth_s = cpool.tile([P, FCH, DPR, 2, P], FP8, name="wth", tag="wth")
            wtl_s = cpool.tile([P, FCH, DPR, 2, P], FP8, name="wtl", tag="wtl")
            wph_s = cpool.tile([P, FCH, DPR, 2, P], FP8, name="wph", tag="wph")
            wpl_s = cpool.tile([P, FCH, DPR, 2, P], FP8, name="wpl", tag="wpl")
            bt_s = cpool.tile([P, FCH], F32, name="bts", tag="bts")
            bp_s = cpool.tile([P, FCH], F32, name="bps", tag="bps")

            def wt_fc(fc):
                return wth_s[:, fc]

            xth_s = [cpool.tile([P, DPR, 2, NF], FP8, name=f"xth{ns}",
                                tag=f"xth{ns}") for ns in range(NSL)]
            xtl_s = [cpool.tile([P, DPR, 2, NF], FP8, name=f"xtl{ns}",
                                tag=f"xtl{ns}") for ns in range(NSL)]

            def xt_h(ns, dp):
                return xth_s[ns][:, dp]

            def xt_l(ns, dp):
                return xtl_s[ns][:, dp]

            # phase-3 fp8 moving streams, one tile per 4 m-pairs
            XJG = 4
            xnh_s = [cpool.tile([P, XJG, 2, D], FP8, name=f"xnh{g}",
                                tag=f"xnh{g}") for g in range(NPR // XJG)]
            xnl_s = [cpool.tile([P, XJG, 2, D], FP8, name=f"xnl{g}",
                                tag=f"xnl{g}") for g in range(NPR // XJG)]

            th_bf = mid.tile([P, FCH, N], BF16, name="thbf")  # thetaT [f, n]
            ph_bf = mid.tile([P, FCH, N], BF16, name="phbf")  # phiT   [f, m]
            # fp8 pair tiles for the score matmuls, tiled per (fpair, ns)
            # so consumers wait only on the two (fc, ns) writes they need
            thh_s = [[mid.tile([P, 2, NF], FP8, name=f"thh{fp}{ns}",
                               tag=f"thh{fp}{ns}") for ns in range(NSL)]
                     for fp in range(FPR)]
            thl_s = [[mid.tile([P, 2, NF], FP8, name=f"thl{fp}{ns}",
                               tag=f"thl{fp}{ns}") for ns in range(NSL)]
                     for fp in range(FPR)]
            phh_s = [[mid.tile([P, 2, NF], FP8, name=f"phh{fp}{ns}",
                               tag=f"phh{fp}{ns}") for ns in range(NSL)]
                     for fp in range(FPR)]
            phl_s = [[mid.tile([P, 2, NF], FP8, name=f"phl{fp}{ns}",
                               tag=f"phl{fp}{ns}") for ns in range(NSL)]
                     for fp in range(FPR)]
            # A = E/s in fp8, pair tiles for DoubleRow stationary operand
            a_s = [apool.tile([P, 2, N], FP8, name=f"as{j}", tag=f"as{j}")
                   for j in range(NPR)]

            xr_r = xr_d[:].rearrange("(c p) d -> p c d", p=P)
            out_r = out_d[:].rearrange("(c p) d -> p c d", p=P)

            # PE warm-up: the clock gate holds PE at 1.2GHz until ~3us of
            # sustained activity; get PE busy ASAP (tiny memset, short
            # 128-free dummies) and keep it busy until the first input
            # tiles land (~2.5us), so the ramp completes early.
            zx = cpool.tile([P, P], BF16, name="zx", tag="zx")
            nc.vector.memset(zx, 0)
            eb_s = cpool.tile([P, 1], F32, name="ebs", tag="ebs")
            nc.vector.memset(eb_s, -20.0)
            zp = psum4.tile([P, NSL, NF], F32, name="pwm", tag="ps4")
            NW = 16
            for i in range(NW):
                nc.tensor.matmul(zp[:, 0, 0:P], zx, zx, start=(i == 0),
                                 stop=(i == NW - 1))

            # --- DMAs in first-use order (HWDGE is one serial FIFO) ---
            # phase-1 matmuls run term-major (T1 all fc, T3, T2) so the
            # DMA arrival order matches consumption: wth+xth, wtl, xtl
            # startup DMAs alternate between the SP and Act queue
            # frontends: descriptor generation (the serial HWDGE stage)
            # for the theta-side and x-side streams overlaps
            nc.sync.dma_start(out=wth_s, in_=wth_d[:])
            nc.scalar.dma_start(out=xth_s[0], in_=xth_d[:, 0])
            nc.sync.dma_start(out=wtl_s, in_=wtl_d[:])
            nc.scalar.dma_start(out=xtl_s[0], in_=xtl_d[:, 0])
            nc.sync.dma_start(out=bt_s, in_=bt_d[:])
            nc.scalar.dma_start(out=xth_s[1], in_=xth_d[:, 1])
            nc.sync.dma_start(out=xtl_s[1], in_=xtl_d[:, 1])
            nc.scalar.dma_start(out=wph_s, in_=wph_d[:])
            nc.sync.dma_start(out=xth_s[2], in_=xth_d[:, 2])
            nc.scalar.dma_start(out=wpl_s, in_=wpl_d[:])
            nc.sync.dma_start(out=xtl_s[2], in_=xtl_d[:, 2])
            nc.scalar.dma_start(out=bp_s, in_=bp_d[:])
            nc.sync.dma_start(out=xth_s[3], in_=xth_d[:, 3])
            nc.sync.dma_start(out=xtl_s[3], in_=xtl_d[:, 3])
            for g in range(NPR // XJG):
                nc.sync.dma_start(out=xnh_s[g],
                                  in_=xnh_d[:, g * XJG:(g + 1) * XJG])
            for g in range(NPR // XJG):
                nc.sync.dma_start(out=xnl_s[g],
                                  in_=xnl_d[:, g * XJG:(g + 1) * XJG])

            # ---------------- Phase 1: projections ----------------
            # thT[f, n] = sigmoid((sum_d 32Wt[d,f] xT[d,n]) / 32 + bt[f])
            for ns in range(NSL):
                nsl = slice(ns * NF, (ns + 1) * NF)
                ps = psum4.tile([P, FCH, NF], F32, name="pth", tag="ps4")
                # term-major so DMA arrivals (wth, xth, wtl, xtl) gate as
                # few matmuls as possible; psum groups interleave by fc
                for t, (w_s, xt) in enumerate(
                        ((None, xt_h), (wtl_s, xt_h), (None, xt_l))):
                    for fc in range(FCH):
                        for dp in range(DPR):
                            lhs = (wt_fc(fc)[:, dp] if w_s is None
                                   else w_s[:, fc, dp])
                            nc.tensor.matmul(
                                ps[:, fc], lhs, xt(ns, dp),
                                start=(t == 0 and dp == 0),
                                stop=(t == 2 and dp == DPR - 1),
                                perf_mode=DR,
                            )
                # all sigmoids first: they are the psum-tile readers, so the
                # slot frees for ns+2 as early as possible
                for fc in range(FCH):
                    nc.scalar.activation(
                        th_bf[:, fc, nsl], ps[:, fc], AF.Sigmoid,
                        bias=bt_s[:, fc:fc + 1], scale=1.0 / WSC,
                    )
                for fc in range(FCH):
                    # hi/lo split for the fp8 score matmuls
                    nc.scalar.activation(
                        thh_s[fc // 2][ns][:, fc % 2], th_bf[:, fc, nsl],
                        AF.Copy,
                    )
                    nc.vector.scalar_tensor_tensor(
                        thl_s[fc // 2][ns][:, fc % 2], th_bf[:, fc, nsl],
                        0.0, thh_s[fc // 2][ns][:, fc % 2],
                        op0=OP.bypass, op1=OP.subtract,
                    )
            sp_queue = []
            for ns in range(NSL):
                nsl = slice(ns * NF, (ns + 1) * NF)
                ps = psum4.tile([P, FCH, NF], F32, name="pph", tag="ps4")
                for t, (w_s, xt) in enumerate(
                        ((wph_s, xt_h), (wpl_s, xt_h), (wph_s, xt_l))):
                    for fc in range(FCH):
                        for dp in range(DPR):
                            nc.tensor.matmul(
                                ps[:, fc], w_s[:, fc, dp], xt(ns, dp),
                                start=(t == 0 and dp == 0),
                                stop=(t == 2 and dp == DPR - 1),
                                perf_mode=DR,
                            )
                # phi bias-add on Act (Identity) keeps DVE free for the lo
                # splits. The ns3 biases are the readers that free the psum
                # slot the second score chunk needs, and Act is busy with
                # exp(mc0) by then — run them split across DVE and Pool,
                # issued BEFORE the ns2 lo-splits so they sit at the queue
                # heads when the phi ns3 matmuls finish.
                if ns < NSL - 1:
                    for fc in range(FCH):
                        nc.scalar.activation(
                            ph_bf[:, fc, nsl], ps[:, fc], AF.Identity,
                            bias=bp_s[:, fc:fc + 1], scale=1.0 / WSC,
                        )
                else:
                    # Pool cannot read PSUM: split DVE/Act instead
                    for fc in range(2):
                        nc.vector.tensor_scalar(
                            ph_bf[:, fc, nsl], ps[:, fc],
                            1.0 / WSC, bp_s[:, fc:fc + 1],
                            op0=OP.mult, op1=OP.add,
                        )
                    for fc in range(2, FCH):
                        nc.scalar.activation(
                            ph_bf[:, fc, nsl], ps[:, fc], AF.Identity,
                            bias=bp_s[:, fc:fc + 1], scale=1.0 / WSC,
                        )
                sp_queue.append(ns)
                if ns == NSL - 2:
                    continue  # ns2 splits issued after the ns3 biases
                for s_ns in sp_queue:
                    s_nsl = slice(s_ns * NF, (s_ns + 1) * NF)
                    for fc in range(FCH):
                        nc.gpsimd.tensor_copy(
                            phh_s[fc // 2][s_ns][:, fc % 2],
                            ph_bf[:, fc, s_nsl],
                        )
                        nc.vector.scalar_tensor_tensor(
                            phl_s[fc // 2][s_ns][:, fc % 2],
                            ph_bf[:, fc, s_nsl],
                            0.0, phh_s[fc // 2][s_ns][:, fc % 2],
                            op0=OP.bypass, op1=OP.subtract,
                        )
                sp_queue.clear()

            # ------------- Phase 2: scores + row softmax -------------
            # ST[m, n] = sum_f phT[f, m] thT[f, n]: 6 DoubleRow matmuls per
            # (m-chunk, ns): ph_h*th_h + ph_l*th_h + ph_h*th_l.
            for mc in range(NCH):
                mns, mo = mc // 4, (mc % 4) * P
                msl = slice(mo, mo + P)
                st = psum4.tile([P, NSL, NF], F32, name="pst", tag="ps4")
                for ns in range(NSL):
                    k = 0
                    for lhs_t, rhs_t in ((phh_s, thh_s), (phl_s, thh_s),
                                         (phh_s, thl_s)):
                        for fp in range(FPR):
                            nc.tensor.matmul(
                                st[:, ns],
                                lhs_t[fp][mns][:, :, msl],
                                rhs_t[fp][ns],
                                start=(k == 0), stop=(k == 5), perf_mode=DR,
                            )
                            k += 1
                e_t = erot.tile([P, N], BF16, name="et", tag="et")
                recip = stats.tile([P, 1], F32, name="recip", tag="recip")
                if mc < NCH - 1:
                    rowsum = stats.tile([P, 1], F32, name="rs", tag="rs")
                    nc.scalar.activation(
                        e_t, st, AF.Exp, bias=eb_s, accum_out=rowsum,
                    )
                    nc.vector.reciprocal(recip, rowsum)
                    eng = nc.vector if mc % 2 == 0 else nc.gpsimd
                    eng.tensor_scalar_mul(a_s[mc // 2][:, mc % 2], e_t, recip)
                else:
                    # last chunk is on the phase-3 critical path: split the
                    # exp into halves (first half overlaps the ns2/3 score
                    # matmuls) and run the two scale halves on DVE + Pool
                    rs2 = stats.tile([P, 2], F32, name="rs2", tag="rs")
                    H = N // 2
                    for h in range(2):
                        nc.scalar.activation(
                            e_t[:, h * H:(h + 1) * H], st[:, 2 * h:2 * h + 2],
                            AF.Exp, bias=eb_s, accum_out=rs2[:, h:h + 1],
                        )
                    rowsum = stats.tile([P, 1], F32, name="rs", tag="rs")
                    nc.vector.reduce_sum(rowsum, rs2, axis=mybir.AxisListType.X)
                    nc.vector.reciprocal(recip, rowsum)
                    nc.vector.tensor_scalar_mul(
                        a_s[mc // 2][:, mc % 2, 0:H], e_t[:, 0:H], recip)
                    nc.gpsimd.tensor_scalar_mul(
                        a_s[mc // 2][:, mc % 2, H:N], e_t[:, H:N], recip)

            # ------------- Phase 3: weighted sum + residual -------------
            # out[n, d] = sum_m A[m, n] (xh[m, d] + xl[m, d]) + x[n, d]
            def p3_mms(groups):
                # groups: list of (psum_target_ap, nch, d_slice). j-outer
                # across all groups of the tile: the last-written a_s pair
                # is only touched near the end, so the phase-2 tail overlaps
                # these matmuls
                for j in range(NPR):        # m-pair
                    for si, xs in enumerate((xnh_s, xnl_s)):
                        for pt, nch, dslc in groups:
                            nc.tensor.matmul(
                                pt,
                                a_s[j][:, :, nch * P:(nch + 1) * P],
                                xs[j // XJG][:, j % XJG, :, dslc],
                                start=(j == 0 and si == 0),
                                stop=(j == NPR - 1 and si == 1),
                                perf_mode=DR,
                            )

            for np_ in range(NPR - 1):      # n-chunk pairs 0..6
                xr_t = xstp.tile([P, 2, D], BF16, name="xrt", tag="xrt")
                nc.sync.dma_start(
                    out=xr_t, in_=xr_r[:, 2 * np_:2 * np_ + 2],
                )
                o_ps = psum4.tile([P, 4, NF], F32, name="po", tag="ps4")
                p3_mms([(o_ps[:, g], 2 * np_ + g // 2,
                         slice((g % 2) * NF, (g % 2 + 1) * NF))
                        for g in range(4)])
                o_sb = ostp.tile([P, 2, D], BF16, name="osb", tag="osb")
                nc.vector.tensor_add(
                    o_sb,
                    o_ps[:].rearrange("p (c s) f -> p c (s f)", c=2),
                    xr_t,
                )
                nc.scalar.dma_start(
                    out=out_r[:, 2 * np_:2 * np_ + 2],
                    in_=o_sb,
                )
            # tail: the last pair runs as progressively smaller passes
            # (chunk 14; then chunk 15 in d-pieces 512/384/128) so the
            # final add+store chain after the last matmul is tiny
            nch14, nch15 = NCH - 2, NCH - 1
            xr14 = xstp.tile([P, 1, D], BF16, name="xr14", tag="xrt")
            nc.sync.dma_start(out=xr14, in_=xr_r[:, nch14:nch14 + 1])
            xr15 = xstp.tile([P, 1, D], BF16, name="xr15", tag="xrt")
            nc.sync.dma_start(out=xr15, in_=xr_r[:, nch15:nch15 + 1])
            o_ps = psum4.tile([P, 2, NF], F32, name="pol", tag="ps4")
            p3_mms([(o_ps[:, dsl], nch14, slice(dsl * NF, (dsl + 1) * NF))
                    for dsl in range(DSL)])
            o_sb = oztp.tile([P, 1, D], BF16, name="osbl", tag="ozs")
            nc.vector.tensor_add(
                o_sb, o_ps[:].rearrange("p (c s) f -> p c (s f)", c=1), xr14,
            )
            nc.scalar.dma_start(out=out_r[:, nch14:nch14 + 1], in_=o_sb)
            pieces = [(0, NF), (NF, NF - P), (2 * NF - P, P)]
            for d0, w in pieces:
                o_ps = psum4.tile([P, 1, w], F32, name=f"pz{d0}", tag="ps4")
                p3_mms([(o_ps[:, 0], nch15, slice(d0, d0 + w))])
                o_sb = oztp.tile([P, 1, w], BF16, name=f"oz{d0}", tag="ozs")
                nc.vector.tensor_add(
                    o_sb, o_ps[:].rearrange("p c f -> p c f"),
                    xr15[:, :, d0:d0 + w],
                )
                nc.scalar.dma_start(
                    out=out_r[:, nch15:nch15 + 1, d0:d0 + w], in_=o_sb)
    nc.finalize()  # Bacc legalization passes (wait splitting, reg alloc, ...)
    return nc


_NC = None


def _get_nc():
    global _NC
    if _NC is None:
        _NC = build_bass()
    return _NC


def make_in_maps(x, Wt, bt, Wp, bp):
    bf16 = ml_dtypes.bfloat16
    e4 = ml_dtypes.float8_e4m3

    def wpair(W):
        # [P, FCH, DPR, 2, P] hi/lo of 32*W
        w = np.asarray(W, np.float64).reshape(DPR, 2, P, FCH, P) * WSC
        w = np.ascontiguousarray(w.transpose(2, 3, 0, 1, 4)).astype(np.float32)
        hi = w.astype(e4)
        lo = (w - hi.astype(np.float32)).astype(e4)
        return hi, lo

    wth, wtl = wpair(Wt)
    wph, wpl = wpair(Wp)
    # bias layout [P, FCH]: bt_r[p, c] = bt[c*P + p]
    fch = bt.size // P
    bt_r = np.ascontiguousarray(np.asarray(bt, np.float32).reshape(fch, P).T)
    bp_r = np.ascontiguousarray(np.asarray(bp, np.float32).reshape(fch, P).T)
    in_maps = []
    for b in range(x.shape[0]):
        xb = np.ascontiguousarray(np.asarray(x[b], np.float32))
        # xT pair layout [P, NSL, DPR, 2, NF]:
        #   [p, ns, dp, i, no] = x[ns*512+no, (2dp+i)*128+p]
        xt = xb.reshape(NSL, NF, DPR, 2, P).transpose(4, 0, 2, 3, 1)
        xt = np.ascontiguousarray(xt)
        xth = xt.astype(e4)
        xtl = (xt - xth.astype(np.float32)).astype(e4)
        # phase-3 pair layout [P, NPR, 2, D]
        xp = xb.reshape(NPR, 2, P, D).transpose(2, 0, 1, 3)
        xnh = xp.astype(e4)
        xnl = (xp - xnh.astype(np.float32)).astype(e4)
        in_maps.append({
            "xth": np.ascontiguousarray(xth),
            "xtl": np.ascontiguousarray(xtl),
            "xnh": np.ascontiguousarray(xnh),
            "xnl": np.ascontiguousarray(xnl),
            "xr": xb.astype(bf16),
            "wth": wth, "wtl": wtl, "wph": wph, "wpl": wpl,
            "bt": bt_r,
            "bp": bp_r,
        })
    return in_maps


def run(inputs, trace=False):
    """Run on 8 NeuronCores; returns (out [B,N,D] f32, BassKernelResults)."""
    x = inputs["x"]
    assert x.shape == (B, N, D), x.shape
    nc = _get_nc()
    in_maps = make_in_maps(x, inputs["Wt"], inputs["bt"], inputs["Wp"], inputs["bp"])
    res = run_bass_kernel_spmd(nc, in_maps, core_ids=list(range(B)), trace=trace)
    out = np.stack([res.results[c]["out"] for c in range(B)], axis=0)
    return out.astype(np.float32), res


def kernel(**inputs) -> np.ndarray:
    out, _ = run(inputs)
    return out


# revision 31
# speedup vs baseline: 1.0011x; 1.0011x over previous
"""Trainium2 Bass kernel for nn_AttentionModule (dense_transformer).

Reference computation (per batch sample b):
    theta = sigmoid(x @ Wt + bt)            # [N, F]
    phi   = x @ Wp + bp                     # [N, F]
    att   = theta @ phi.T                   # [N(n), N(m)]
    att   = softmax(att, axis over n)       # softmax over QUERY axis
    out   = att(n,m) @ x(m,d) + x           # [N, D]
  (the g = tanh(x@Wg+bg) branch is dead — never used in the output)

Strategy: pure data parallelism — B=8 samples, one per NeuronCore. No
collectives. Per core, everything runs in transposed score layout
ST[m, n] = phi[m]·theta[n] so the softmax axis (n) is the free axis;
softmax runs without max-subtraction (logits < ~60, exp(ST-20) is
fp32-safe and the shift cancels in the normalization).

ALL matmuls use fp8e4 DoubleRow (0.5 PE cycles/output column — 4x the
bf16 rate; each instruction contracts a pair of 128-deep k-tiles).
Accuracy comes from hi+lo operand splitting (x_lo = x - fp8(x)):
 - projections: 3 terms (Wh·xh + Wh·xl + Wl·xh), W pre-scaled by 32 on
   the host so W*32 ~ N(0,1) avoids the fp8 subnormal floor; the /32
   rides the activation `scale` input. Splits are free (host-side).
 - scores: 3 terms with theta/phi split on-device (Act/Pool copy for
   hi, DVE scalar_tensor_tensor for lo), tiled per (fc, ns) so the
   splits pipeline behind the projections.
 - phase 3: A = E/s quantized to single fp8 (~1.3e-2, the dominant
   error term), x split hi+lo on the host (2 terms).
 - residual in bf16, output stored bf16 (harness converts to f32).
Measured end-to-end rel err ~1.4e-2 (gate: 2e-2), on hw and CoreSim.

Phase structure per core:
 P1: thetaT/phiT [f, n]: per (ns, fc): 12 DoubleRow matmuls into one
     bank of a 4-bank PSUM tile; sigmoid (scale=1/32, bias bt) / DVE
     tensor_scalar (mult 1/32, add bp); then per (fc, ns) hi/lo split
     ops writing fp8 pair tiles.
 P2: per m-chunk: 24 DoubleRow matmuls -> ST [128, 2048] in 4 PSUM
     banks; ONE exp activation (bias -20) -> E bf16 rotating tile,
     accum_out gives the row-sum free; reciprocal; tensor_scalar_mul
     E*(1/s) -> A fp8 pair tiles (alternating DVE/Pool).
 P3: per n-chunk-pair: 4 accumulation groups in one 4-bank PSUM tile,
     16 DoubleRow matmuls each (8 m-pairs x {xn_hi, xn_lo}); DVE adds
     the bf16 residual; DMA out bf16.

Scheduling notes (walrus sync-wait limits + Tile dep granularity):
 - built as bacc.Bacc: finalize() legalizes multi-sem waits;
 - every SBUF tile is written by exactly ONE dma_start, and tiles are
   split to match consumer granularity (deps are tile-granular);
 - SBUF pools never overlap/reuse address space;
 - PE warm-up: dummy matmuls burn the initial DMA-wait so the real
   stream starts at 2.4GHz.
"""

import numpy as np
import ml_dtypes

import concourse.bass as bass
import concourse.bacc as bacc
import concourse.mybir as mybir
from concourse.tile import TileContext
from concourse.bass_utils import run_bass_kernel_spmd

P = 128
B, N, D, F = 8, 2048, 1024, 512
NCH = N // P    # 16 chunks of the token dim
NPR = NCH // 2  # 8 m-chunk pairs (DoubleRow granularity)
DCH = D // P    # 8 chunks of the model dim
DPR = DCH // 2  # 4 d-chunk pairs
FCH = F // P    # 4 chunks of the filter dim
FPR = FCH // 2  # 2 f-chunk pairs
NF = 512        # matmul moving free dim (one fp32 PSUM bank)
NSL = N // NF   # 4 score column slices
DSL = D // NF   # 2 output d slices
WSC = 32.0      # host pre-scale of W (keeps W*32 out of fp8 subnormals)

BF16 = mybir.dt.bfloat16
FP8 = mybir.dt.float8e4
F32 = mybir.dt.float32
AF = mybir.ActivationFunctionType
OP = mybir.AluOpType
DR = mybir.MatmulPerfMode.DoubleRow


def build_bass():
    nc = bacc.Bacc()

    # x.T in DoubleRow pair layout, hi/lo fp8 streams, ns-major for
    # contiguous per-ns DMAs: [p, ns, dp, i, no] = x[ns*512+no, (2dp+i)*128+p]
    xth_d = nc.declare_dram_parameter("xth", [P, NSL, DPR, 2, NF], FP8, isOutput=False)
    xtl_d = nc.declare_dram_parameter("xtl", [P, NSL, DPR, 2, NF], FP8, isOutput=False)
    # phase-3 moving streams: [p, j, i, d] = x[(2j+i)*128+p, d]
    xnh_d = nc.declare_dram_parameter("xnh", [P, NPR, 2, D], FP8, isOutput=False)
    xnl_d = nc.declare_dram_parameter("xnl", [P, NPR, 2, D], FP8, isOutput=False)
    xr_d = nc.declare_dram_parameter("xr", [N, D], BF16, isOutput=False)
    # weights (pre-scaled by 32) in pair layout:
    # [p, fc, dp, i, fo] = 32*W[(2dp+i)*128+p, fc*128+fo]
    wth_d = nc.declare_dram_parameter("wth", [P, FCH, DPR, 2, P], FP8, isOutput=False)
    wtl_d = nc.declare_dram_parameter("wtl", [P, FCH, DPR, 2, P], FP8, isOutput=False)
    wph_d = nc.declare_dram_parameter("wph", [P, FCH, DPR, 2, P], FP8, isOutput=False)
    wpl_d = nc.declare_dram_parameter("wpl", [P, FCH, DPR, 2, P], FP8, isOutput=False)
    bt_d = nc.declare_dram_parameter("bt", [P, FCH], F32, isOutput=False)
    bp_d = nc.declare_dram_parameter("bp", [P, FCH], F32, isOutput=False)
    out_d = nc.declare_dram_parameter("out", [N, D], BF16, isOutput=True)

    with TileContext(nc) as tc:
        with (
            tc.tile_pool(name="const", bufs=1) as cpool,
            tc.tile_pool(name="mid", bufs=1) as mid,
            tc.tile_pool(name="apool", bufs=1) as apool,
            tc.tile_pool(name="erot", bufs=2) as erot,
            tc.tile_pool(name="stats", bufs=16) as stats,
            tc.tile_pool(name="xst", bufs=2) as xstp,
            tc.tile_pool(name="ost", bufs=2) as ostp,
            tc.tile_pool(name="ozt", bufs=3) as oztp,
            tc.tile_pool(name="psum4", bufs=2, space="PSUM") as psum4,
        ):
            # --- constant/streamed input tiles (one DMA each) ---
            wth0_s = cpool.tile([P, DPR, 2, P], FP8, name="wth0", tag="wth0")
            wthK_s = cpool.tile([P, FCH - 1, DPR, 2, P], FP8, name="wthk", tag="wthk")
            wtl_s = cpool.tile([P, FCH, DPR, 2, P], FP8, name="wtl", tag="wtl")
            wph_s = cpool.tile([P, FCH, DPR, 2, P], FP8, name="wph", tag="wph")
            wpl_s = cpool.tile([P, FCH, DPR, 2, P], FP8, name="wpl", tag="wpl")
            bt_s = cpool.tile([P, FCH], F32, name="bts", tag="bts")
            bp_s = cpool.tile([P, FCH], F32, name="bps", tag="bps")

            def wt_fc(fc):
                return wth0_s if fc == 0 else wthK_s[:, fc - 1]

            # xT streams: ns0 split in dp-halves (startup-critical)
            xth0_s = [cpool.tile([P, 2, 2, NF], FP8, name=f"xth0{h}",
                                 tag=f"xth0{h}") for h in range(2)]
            xth_s = [None] + [cpool.tile([P, DPR, 2, NF], FP8, name=f"xth{ns}",
                                         tag=f"xth{ns}") for ns in range(1, NSL)]
            xtl_s = [cpool.tile([P, DPR, 2, NF], FP8, name=f"xtl{ns}",
                                tag=f"xtl{ns}") for ns in range(NSL)]

            def xt_h(ns, dp):
                if ns == 0:
                    return xth0_s[dp // 2][:, dp % 2]
                return xth_s[ns][:, dp]

            def xt_l(ns, dp):
                return xtl_s[ns][:, dp]

            # phase-3 fp8 moving streams, one tile per 4 m-pairs
            XJG = 4
            xnh_s = [cpool.tile([P, XJG, 2, D], FP8, name=f"xnh{g}",
                                tag=f"xnh{g}") for g in range(NPR // XJG)]
            xnl_s = [cpool.tile([P, XJG, 2, D], FP8, name=f"xnl{g}",
                                tag=f"xnl{g}") for g in range(NPR // XJG)]

            th_bf = mid.tile([P, FCH, N], BF16, name="thbf")  # thetaT [f, n]
            ph_bf = mid.tile([P, FCH, N], BF16, name="phbf")  # phiT   [f, m]
            # fp8 pair tiles for the score matmuls, tiled per (fpair, ns)
            # so consumers wait only on the two (fc, ns) writes they need
            thh_s = [[mid.tile([P, 2, NF], FP8, name=f"thh{fp}{ns}",
                               tag=f"thh{fp}{ns}") for ns in range(NSL)]
                     for fp in range(FPR)]
            thl_s = [[mid.tile([P, 2, NF], FP8, name=f"thl{fp}{ns}",
                               tag=f"thl{fp}{ns}") for ns in range(NSL)]
                     for fp in range(FPR)]
            phh_s = [[mid.tile([P, 2, NF], FP8, name=f"phh{fp}{ns}",
                               tag=f"phh{fp}{ns}") for ns in range(NSL)]
                     for fp in range(FPR)]
            phl_s = [[mid.tile([P, 2, NF], FP8, name=f"phl{fp}{ns}",
                               tag=f"phl{fp}{ns}") for ns in range(NSL)]
                     for fp in range(FPR)]
            # A = E/s in fp8, pair tiles for DoubleRow stationary operand
            a_s = [apool.tile([P, 2, N], FP8, name=f"as{j}", tag=f"as{j}")
                   for j in range(NPR)]

            xr_r = xr_d[:].rearrange("(c p) d -> p c d", p=P)
            out_r = out_d[:].rearrange("(c p) d -> p c d", p=P)

            # PE warm-up: the clock gate holds PE at 1.2GHz until ~3us of
            # sustained activity; get PE busy ASAP (tiny memset, short
            # 128-free dummies) and keep it busy until the first input
            # tiles land (~2.5us), so the ramp completes early.
            zx = cpool.tile([P, P], BF16, name="zx", tag="zx")
            nc.vector.memset(zx, 0)
            eb_s = cpool.tile([P, 1], F32, name="ebs", tag="ebs")
            nc.vector.memset(eb_s, -20.0)
            zp = psum4.tile([P, NSL, NF], F32, name="pwm", tag="ps4")
            NW = 16
            for i in range(NW):
                nc.tensor.matmul(zp[:, 0, 0:P], zx, zx, start=(i == 0),
                                 stop=(i == NW - 1))

            # --- DMAs in first-use order (HWDGE is one serial FIFO) ---
            # phase-1 matmuls run term-major (T1 all fc, T3, T2) so the
            # DMA arrival order matches consumption: wth+xth, wtl, xtl
            # startup DMAs alternate between the SP and Act queue
            # frontends: descriptor generation (the serial HWDGE stage)
            # for the theta-side and x-side streams overlaps
            nc.sync.dma_start(out=wth0_s, in_=wth_d[:, 0])
            nc.scalar.dma_start(out=xth0_s[0], in_=xth_d[:, 0, 0:2])
            nc.sync.dma_start(out=xth0_s[1], in_=xth_d[:, 0, 2:4])
            nc.scalar.dma_start(out=wthK_s, in_=wth_d[:, 1:FCH])
            nc.sync.dma_start(out=wtl_s, in_=wtl_d[:])
            nc.scalar.dma_start(out=xtl_s[0], in_=xtl_d[:, 0])
            nc.sync.dma_start(out=bt_s, in_=bt_d[:])
            nc.scalar.dma_start(out=xth_s[1], in_=xth_d[:, 1])
            nc.sync.dma_start(out=xtl_s[1], in_=xtl_d[:, 1])
            nc.scalar.dma_start(out=wph_s, in_=wph_d[:])
            nc.sync.dma_start(out=xth_s[2], in_=xth_d[:, 2])
            nc.scalar.dma_start(out=wpl_s, in_=wpl_d[:])
            nc.sync.dma_start(out=xtl_s[2], in_=xtl_d[:, 2])
            nc.scalar.dma_start(out=bp_s, in_=bp_d[:])
            nc.sync.dma_start(out=xth_s[3], in_=xth_d[:, 3])
            nc.sync.dma_start(out=xtl_s[3], in_=xtl_d[:, 3])
            for g in range(NPR // XJG):
                nc.sync.dma_start(out=xnh_s[g],
                                  in_=xnh_d[:, g * XJG:(g + 1) * XJG])
            for g in range(NPR // XJG):
                nc.sync.dma_start(out=xnl_s[g],
                                  in_=xnl_d[:, g * XJG:(g + 1) * XJG])

            # ---------------- Phase 1: projections ----------------
            # thT[f, n] = sigmoid((sum_d 32Wt[d,f] xT[d,n]) / 32 + bt[f])
            for ns in range(NSL):
                nsl = slice(ns * NF, (ns + 1) * NF)
                ps = psum4.tile([P, FCH, NF], F32, name="pth", tag="ps4")
                # term-major so DMA arrivals (wth, xth, wtl, xtl) gate as
                # few matmuls as possible; psum groups interleave by fc
                for t, (w_s, xt) in enumerate(
                        ((None, xt_h), (wtl_s, xt_h), (None, xt_l))):
                    for fc in range(FCH):
                        for dp in range(DPR):
                            lhs = (wt_fc(fc)[:, dp] if w_s is None
                                   else w_s[:, fc, dp])
                            nc.tensor.matmul(
                                ps[:, fc], lhs, xt(ns, dp),
                                start=(t == 0 and dp == 0),
                                stop=(t == 2 and dp == DPR - 1),
                                perf_mode=DR,
                            )
                # all sigmoids first: they are the psum-tile readers, so the
                # slot frees for ns+2 as early as possible
                for fc in range(FCH):
                    nc.scalar.activation(
                        th_bf[:, fc, nsl], ps[:, fc], AF.Sigmoid,
                        bias=bt_s[:, fc:fc + 1], scale=1.0 / WSC,
                    )
                for fc in range(FCH):
                    # hi/lo split for the fp8 score matmuls
                    nc.scalar.activation(
                        thh_s[fc // 2][ns][:, fc % 2], th_bf[:, fc, nsl],
                        AF.Copy,
                    )
                    nc.vector.scalar_tensor_tensor(
                        thl_s[fc // 2][ns][:, fc % 2], th_bf[:, fc, nsl],
                        0.0, thh_s[fc // 2][ns][:, fc % 2],
                        op0=OP.bypass, op1=OP.subtract,
                    )
            sp_queue = []
            for ns in range(NSL):
                nsl = slice(ns * NF, (ns + 1) * NF)
                ps = psum4.tile([P, FCH, NF], F32, name="pph", tag="ps4")
                for t, (w_s, xt) in enumerate(
                        ((wph_s, xt_h), (wpl_s, xt_h), (wph_s, xt_l))):
                    for fc in range(FCH):
                        for dp in range(DPR):
                            nc.tensor.matmul(
                                ps[:, fc], w_s[:, fc, dp], xt(ns, dp),
                                start=(t == 0 and dp == 0),
                                stop=(t == 2 and dp == DPR - 1),
                                perf_mode=DR,
                            )
                # phi bias-add on Act (Identity) keeps DVE free for the lo
                # splits. The ns3 biases are the readers that free the psum
                # slot the second score chunk needs, and Act is busy with
                # exp(mc0) by then — run them split across DVE and Pool,
                # issued BEFORE the ns2 lo-splits so they sit at the queue
                # heads when the phi ns3 matmuls finish.
                if ns < NSL - 1:
                    for fc in range(FCH):
                        nc.scalar.activation(
                            ph_bf[:, fc, nsl], ps[:, fc], AF.Identity,
                            bias=bp_s[:, fc:fc + 1], scale=1.0 / WSC,
                        )
                else:
                    # Pool cannot read PSUM: split DVE/Act instead
                    for fc in range(2):
                        nc.vector.tensor_scalar(
                            ph_bf[:, fc, nsl], ps[:, fc],
                            1.0 / WSC, bp_s[:, fc:fc + 1],
                            op0=OP.mult, op1=OP.add,
                        )
                    for fc in range(2, FCH):
                        nc.scalar.activation(
                            ph_bf[:, fc, nsl], ps[:, fc], AF.Identity,
                            bias=bp_s[:, fc:fc + 1], scale=1.0 / WSC,
                        )
                sp_queue.append(ns)
                if ns == NSL - 2:
                    continue  # ns2 splits issued after the ns3 biases
                for s_ns in sp_queue:
                    s_nsl = slice(s_ns * NF, (s_ns + 1) * NF)
                    for fc in range(FCH):
                        nc.gpsimd.tensor_copy(
                            phh_s[fc // 2][s_ns][:, fc % 2],
                            ph_bf[:, fc, s_nsl],
                        )
                        nc.vector.scalar_tensor_tensor(
                            phl_s[fc // 2][s_ns][:, fc % 2],
                            ph_bf[:, fc, s_nsl],
                            0.0, phh_s[fc // 2][s_ns][:, fc % 2],
                            op0=OP.bypass, op1=OP.subtract,
                        )
                sp_queue.clear()

            # ------------- Phase 2: scores + row softmax -------------
            # ST[m, n] = sum_f phT[f, m] thT[f, n]: 6 DoubleRow matmuls per
            # (m-chunk, ns): ph_h*th_h + ph_l*th_h + ph_h*th_l.
            for mc in range(NCH):
                mns, mo = mc // 4, (mc % 4) * P
                msl = slice(mo, mo + P)
                st = psum4.tile([P, NSL, NF], F32, name="pst", tag="ps4")
                for ns in range(NSL):
                    k = 0
                    for lhs_t, rhs_t in ((phh_s, thh_s), (phl_s, thh_s),
                                         (phh_s, thl_s)):
                        for fp in range(FPR):
                            nc.tensor.matmul(
                                st[:, ns],
                                lhs_t[fp][mns][:, :, msl],
                                rhs_t[fp][ns],
                                start=(k == 0), stop=(k == 5), perf_mode=DR,
                            )
                            k += 1
                e_t = erot.tile([P, N], BF16, name="et", tag="et")
                recip = stats.tile([P, 1], F32, name="recip", tag="recip")
                if mc < NCH - 1:
                    rowsum = stats.tile([P, 1], F32, name="rs", tag="rs")
                    nc.scalar.activation(
                        e_t, st, AF.Exp, bias=eb_s, accum_out=rowsum,
                    )
                    nc.vector.reciprocal(recip, rowsum)
                    eng = nc.vector if mc % 2 == 0 else nc.gpsimd
                    eng.tensor_scalar_mul(a_s[mc // 2][:, mc % 2], e_t, recip)
                else:
                    # last chunk is on the phase-3 critical path: split the
                    # exp into halves (first half overlaps the ns2/3 score
                    # matmuls) and run the two scale halves on DVE + Pool
                    rs2 = stats.tile([P, 2], F32, name="rs2", tag="rs")
                    H = N // 2
                    for h in range(2):
                        nc.scalar.activation(
                            e_t[:, h * H:(h + 1) * H], st[:, 2 * h:2 * h + 2],
                            AF.Exp, bias=eb_s, accum_out=rs2[:, h:h + 1],
                        )
                    rowsum = stats.tile([P, 1], F32, name="rs", tag="rs")
                    nc.vector.reduce_sum(rowsum, rs2, axis=mybir.AxisListType.X)
                    nc.vector.reciprocal(recip, rowsum)
                    nc.vector.tensor_scalar_mul(
                        a_s[mc // 2][:, mc % 2, 0:H], e_t[:, 0:H], recip)
                    nc.gpsimd.tensor_scalar_mul(
                        a_s[mc // 2][:, mc % 2, H:N], e_t[:, H:N], recip)

            # ------------- Phase 3: weighted sum + residual -------------
            # out[n, d] = sum_m A[m, n] (xh[m, d] + xl[m, d]) + x[n, d]
            def p3_mms(groups):
                # groups: list of (psum_target_ap, nch, d_slice). j-outer
                # across all groups of the tile: the last-written a_s pair
                # is only touched near the end, so the phase-2 tail overlaps
                # these matmuls
                for j in range(NPR):        # m-pair
                    for si, xs in enumerate((xnh_s, xnl_s)):
                        for pt, nch, dslc in groups:
                            nc.tensor.matmul(
                                pt,
                                a_s[j][:, :, nch * P:(nch + 1) * P],
                                xs[j // XJG][:, j % XJG, :, dslc],
                                start=(j == 0 and si == 0),
                                stop=(j == NPR - 1 and si == 1),
                                perf_mode=DR,
                            )

            for np_ in range(NPR - 1):      # n-chunk pairs 0..6
                xr_t = xstp.tile([P, 2, D], BF16, name="xrt", tag="xrt")
                nc.sync.dma_start(
                    out=xr_t, in_=xr_r[:, 2 * np_:2 * np_ + 2],
                )
                o_ps = psum4.tile([P, 4, NF], F32, name="po", tag="ps4")
                p3_mms([(o_ps[:, g], 2 * np_ + g // 2,
                         slice((g % 2) * NF, (g % 2 + 1) * NF))
                        for g in range(4)])
                o_sb = ostp.tile([P, 2, D], BF16, name="osb", tag="osb")
                nc.vector.tensor_add(
                    o_sb,
                    o_ps[:].rearrange("p (c s) f -> p c (s f)", c=2),
                    xr_t,
                )
                nc.scalar.dma_start(
                    out=out_r[:, 2 * np_:2 * np_ + 2],
                    in_=o_sb,
                )
            # tail: the last pair runs as progressively smaller passes
            # (chunk 14; then chunk 15 in d-pieces 512/384/128) so the
            # final add+store chain after the last matmul is tiny
            nch14, nch15 = NCH - 2, NCH - 1
            xr14 = xstp.tile([P, 1, D], BF16, name="xr14", tag="xrt")
            nc.sync.dma_start(out=xr14, in_=xr_r[:, nch14:nch14 + 1])
            xr15 = xstp.tile([P, 1, D], BF16, name="xr15", tag="xrt")
            nc.sync.dma_start(out=xr15, in_=xr_r[:, nch15:nch15 + 1])
            o_ps = psum4.tile([P, 2, NF], F32, name="pol", tag="ps4")
            p3_mms([(o_ps[:, dsl], nch14, slice(dsl * NF, (dsl + 1) * NF))
                    for dsl in range(DSL)])
            o_sb = oztp.tile([P, 1, D], BF16, name="osbl", tag="ozs")
            nc.vector.tensor_add(
                o_sb, o_ps[:].rearrange("p (c s) f -> p c (s f)", c=1), xr14,
            )
            nc.scalar.dma_start(out=out_r[:, nch14:nch14 + 1], in_=o_sb)
            pieces = [(0, NF), (NF, NF - P), (2 * NF - P, P)]
            for d0, w in pieces:
                o_ps = psum4.tile([P, 1, w], F32, name=f"pz{d0}", tag="ps4")
                p3_mms([(o_ps[:, 0], nch15, slice(d0, d0 + w))])
                o_sb = oztp.tile([P, 1, w], BF16, name=f"oz{d0}", tag="ozs")
                nc.vector.tensor_add(
                    o_sb, o_ps[:].rearrange("p c f -> p c f"),
                    xr15[:, :, d0:d0 + w],
                )
                nc.scalar.dma_start(
                    out=out_r[:, nch15:nch15 + 1, d0:d0 + w], in_=o_sb)
    nc.finalize()  # Bacc legalization passes (wait splitting, reg alloc, ...)
    return nc


_NC = None


def _get_nc():
    global _NC
    if _NC is None:
        _NC = build_bass()
    return _NC


def make_in_maps(x, Wt, bt, Wp, bp):
    bf16 = ml_dtypes.bfloat16
    e4 = ml_dtypes.float8_e4m3

    def wpair(W):
        # [P, FCH, DPR, 2, P] hi/lo of 32*W
        w = np.asarray(W, np.float64).reshape(DPR, 2, P, FCH, P) * WSC
        w = np.ascontiguousarray(w.transpose(2, 3, 0, 1, 4)).astype(np.float32)
        hi = w.astype(e4)
        lo = (w - hi.astype(np.float32)).astype(e4)
        return hi, lo

    wth, wtl = wpair(Wt)
    wph, wpl = wpair(Wp)
    # bias layout [P, FCH]: bt_r[p, c] = bt[c*P + p]
    fch = bt.size // P
    bt_r = np.ascontiguousarray(np.asarray(bt, np.float32).reshape(fch, P).T)
    bp_r = np.ascontiguousarray(np.asarray(bp, np.float32).reshape(fch, P).T)
    in_maps = []
    for b in range(x.shape[0]):
        xb = np.ascontiguousarray(np.asarray(x[b], np.float32))
        # xT pair layout [P, NSL, DPR, 2, NF]:
        #   [p, ns, dp, i, no] = x[ns*512+no, (2dp+i)*128+p]
        xt = xb.reshape(NSL, NF, DPR, 2, P).transpose(4, 0, 2, 3, 1)
        xt = np.ascontiguousarray(xt)
        xth = xt.astype(e4)
        xtl = (xt - xth.astype(np.float32)).astype(e4)
        # phase-3 pair layout [P, NPR, 2, D]
        xp = xb.reshape(NPR, 2, P, D).transpose(2, 0, 1, 3)
        xnh = xp.astype(e4)
        xnl = (xp - xnh.astype(np.float32)).astype(e4)
        in_maps.append({
            "xth": np.ascontiguousarray(xth),
            "xtl": np.ascontiguousarray(xtl),
            "xnh": np.ascontiguousarray(xnh),
            "xnl": np.ascontiguousarray(xnl),
            "xr": xb.astype(bf16),
            "wth": wth, "wtl": wtl, "wph": wph, "wpl": wpl,
            "bt": bt_r,
            "bp": bp_r,
        })
    return in_maps


def run(inputs, trace=False):
    """Run on 8 NeuronCores; returns (out [B,N,D] f32, BassKernelResults)."""
    x = inputs["x"]
    assert x.shape == (B, N, D), x.shape
    nc = _get_nc()
    in_maps = make_in_maps(x, inputs["Wt"], inputs["bt"], inputs["Wp"], inputs["bp"])
    res = run_bass_kernel_spmd(nc, in_maps, core_ids=list(range(B)), trace=trace)
    out = np.stack([res.results[c]["out"] for c in range(B)], axis=0)
    return out.astype(np.float32), res


def kernel(**inputs) -> np.ndarray:
    out, _ = run(inputs)
    return out
